# revision 51
# baseline (speedup 1.0000x reference)
"""LocalMean 5x5 box filter (reflect pad) on TRN2, data-parallel over 8 cores.

Full input:  image (32, 3, 512, 512) fp32
Full output: same shape, 5x5 mean with reflect padding on H and W.

Sharding: batch dim 32 -> 4 images per core (12 channel planes of 512x512).

Design (each step HW-profiled on this fleet; 641us -> ~69us; run-to-run
spread at full clock ~±1.5us, occasional chip-wide P0/thermal throttle
adds ~10%):

Session-2 optimizations on top of the v1 pipeline below (76us -> ~69us):
  - Loads merged: the subtile bases (0,124,248,372) are uniform
    stride-124, so ONE 3-D-AP SWDGE casting DMA covers the 4 main
    subtiles (plus the 16-row tail DMA). This matters because SWDGE
    descriptor generation (DIRECT2D) costs ~0.65-0.7us of serial Q7 time
    per dma_start: with 5 loads/plane (60 total) desc-gen alone was
    ~40us, pacing the 36us load stream. With 2/plane it drops to ~17us
    and the stream runs at the HBM limit. (A 2-plane-per-DMA merge is
    impossible: DMA APs cap at 3 dims.)
  - Stores split 2+1 per plane ([0:126, 0:2W], [0:126, 2W:4W], out2) so
    the store stream starts after evac g1 and stays dense — merged
    per-plane stores starve the SDMA ring at the end (the drain is
    store-backlog-bound: at last-load time ~5us of store data is still
    queued; it flushes while the last plane computes, so shortening the
    last plane's compute chain does NOT move the end — measured).
  - Planes 0..9 use whole-plane DVE ops (lowest DVE busy ~40us; DVE is
    the second-binding engine and under chip throttle approaches
    co-bottleneck with DMA); the last two planes are chunked into row
    groups {0,1}/{2,3,4} and the last plane's main load is split in two,
    trimming the pipeline-drain tail.
  - xh_bufs=8 / out_bufs=5: deeper prefetch + store queues so the load
    stream never stalls on WAR reuse when compute lags (helps most when
    the chip is throttled).
  - Cross-engine "help" in the tail (evacs on DVE, stores on the GpSimd
    ring) measurably HURTS: engine queues are strict FIFO, so inserted
    ops serialize against the chain that feeds them (+3us). Don't.
  - shared_tail=True (one early 384KB load for all 12 planes' tail rows
    + piece-wise tail adds in the DVE startup window) also measured
    neutral-to-worse (+0.1..1us): the early 16-partition load and the
    mid-stream DVE insertions cost what the 12 small DMAs saved.
  - Startup is ~9us of fixed cost (NEFF init barrier ~3us, per-engine
    TENSOR_LOAD ~1.2us, bass preamble sem-clears + all-engine barrier,
    first desc-gen) — user instructions cannot start before the bass
    preamble barrier, so prologue-DMA tricks don't apply. Epilogue
    (scope-end semaphore joins) is ~2.5us after the last DMA.
  - Net: exec ~= 9us startup + ~56.5us DMA stream (19.4MB at the
    ~358GB/s HBM limit + ~4% overhead, loads and stores on separate
    rings saturating the union) + ~3us drain/epilogue. The kernel is at
    the HBM floor; remaining headroom is almost entirely framework
    startup/epilogue.

v1 pipeline (641us -> 75us):
  - Single fp16 pipeline (X in [0,1): fp16 round-off ~2^-12; end-to-end
    rel err ~1e-3 incl. fp16 output store, well under the 2e-2 gate).
    This replaces v4's exact bf16+fp16 split, whose gpsimd
    tensor_scalar (+32768 fixed-grid) measured 18.4us/instr here.
  - fp32->fp16 conversion happens INSIDE the load DMAs (SWDGE casts
    inline; HWDGE cannot cast). This removes the DVE cast — the only
    2-port-perf-mode DVE op — which matters because GpSimd/SWDGE
    activity completely stalls DVE 2-port ops (shared SBUF port;
    HW-measured: a 160ns pad copy stretched to 4.4us under a GpSimd
    op). All remaining DVE ops are 1-port fp16 2x_1P.
  - Horizontal 5-tap: A[w] = Xp[w] + Xp[w+2] and P[w] = A[w] + Xp[w+4]
    on DVE (both 4B-aligned fp16 2x packed adds); the remaining taps
    A[w+1] stream into the PE as a second accumulating matmul (the PE
    has no rhs alignment penalty; a DVE op on the misaligned operand
    would drop to 1x mode).
  - Vertical 5-tap via band-matrix matmul (V in {0,1,2}, fp16 exact):
    PSUM = V^T @ P + V^T @ A<<1. M=128 full columns so every PSUM
    partition is written (evac+store never read uninitialized memory).
    2 matmuls per row group x 5 groups = 10 per plane (v4: 30).
  - PSUM evacuation on ScalarE with the 1/25 scale, f16 output.
  - Stores ride the Sync HWDGE ring, which carries ONLY stores (loads
    own the SWDGE ring, so the two DMA streams overlap; sharing a ring
    head-of-line-blocked loads behind store semaphores, +18us). Three
    compact f16 stores per plane — [0:126] of chunks 0-2 (ready after
    evac g2), [0:126] of chunk 3, and [0:14] of the tail chunk — trim the ~20% garbage partitions a full
    128-partition store would carry (~1.3MB/core less HBM write
    traffic). Each store depends on exact whole-evacuation outputs
    (partial-range reads of fused evacuations correlated with an
    intermittent correctness failure in an earlier variant). Host
    reassembles rows and upcasts to f32.
  - Tail rows (496-511) live in partitions 0-15 of subtile 4 of the
    same tiles, so every elementwise op covers them for free.
  - With this structure the kernel sits essentially at the HBM floor:
    DMA union busy ~= (12.97MB fp32 reads + 6.5MB f16 writes) / 358
    GB/s; residual +-4us run spread comes from free-running HAM phase.
"""

import numpy as np

import concourse.bass as bass
import concourse.mybir as mybir
import concourse.tile as tile
from concourse.tile import add_dep_helper
from concourse.bass_utils import run_bass_kernel_spmd

try:
    from bass_rust import AP as RustAP
except ImportError:  # pragma: no cover
    RustAP = None

F32 = mybir.dt.float32
F16 = mybir.dt.float16

N_CORES = 8
NB = 32
NBPC = NB // N_CORES
NCH = NBPC * 3
H = W = 512
PATCH = 5
PAD = 2
INV_AREA = 1.0 / float(PATCH * PATCH)

# Row groups: (in_base, K, out_base, M)
GROUPS = [
    (0, 128, 0, 126),
    (124, 128, 126, 124),
    (248, 128, 250, 124),
    (372, 128, 374, 124),
    (496, 16, 498, 14),
]
XTW = W + 2 * PAD  # 516 padded width
NSUB = 5  # 4 main 128-row subtiles + tail rows in partitions 0-15 of subtile 4


def _reflect(t, n):
    if t < 0:
        t = -t
    if t > n - 1:
        t = 2 * (n - 1) - t
    return t


def _v_matrix(in_base, k_rows, out_base, m_rows):
    v = np.zeros((128, 128), np.float32)
    for m in range(m_rows):
        r = out_base + m
        for t in range(r - PAD, r + PAD + 1):
            k = _reflect(t, H) - in_base
            assert 0 <= k < k_rows, (r, t, k)
            v[k, m] += 1.0
    return v


def _build_vmats():
    v = np.stack(
        [
            _v_matrix(*GROUPS[0]),
            _v_matrix(*GROUPS[1]),
            _v_matrix(*GROUPS[4]),
        ]
    )
    assert np.all(np.isin(v, [0.0, 1.0, 2.0]))
    return v


VMATS16 = _build_vmats().astype(np.float16)
_VM_IDX = [0, 1, 1, 1, 2]


def _mk_ap(like_ap, offset, pattern):
    return RustAP(tensor=like_ap.tensor, offset=offset, ap=pattern)


def build_module(
    split_waits=True,
    tail_opt=False,
    whole_planes=10,
    psum_bufs=8,
    merged_tail=False,
    tail3=False,
    xh_bufs=8,
    out_bufs=5,
    shared_tail=False,
):
    assert not (shared_tail and (tail3 or tail_opt))
    nc = bass.Bass()
    img = nc.dram_tensor("image", [NCH, H, W], F32, kind="ExternalInput")
    vm16 = nc.dram_tensor("vmats16", [3, 128, 128], F16, kind="ExternalInput")
    out1 = nc.dram_tensor("out1", [NCH, 126, 4 * W], F16, kind="ExternalOutput")
    if merged_tail:
        out2 = nc.dram_tensor("out2", [14, NCH * W], F16, kind="ExternalOutput")
    else:
        out2 = nc.dram_tensor("out2", [NCH, 14, W], F16, kind="ExternalOutput")

    with tile.TileContext(nc) as tc:
        with (
            tc.tile_pool(name="const", bufs=1) as constp,
            tc.tile_pool(name="xh", bufs=xh_bufs) as xhp,
            tc.tile_pool(
                name="psum", bufs=psum_bufs, space=bass.MemorySpace.PSUM
            ) as psump,
            tc.tile_pool(name="outp", bufs=out_bufs) as outp,
        ):
            otail = (
                constp.tile([128, NCH * W], F16, name="otail")
                if merged_tail
                else None
            )
            vt16 = constp.tile([128, 3 * 128], F16)
            vt16r = vt16[:].rearrange("p (i m) -> p i m", i=3)
            nc.sync.dma_start(
                vt16r, _mk_ap(vm16[:], 0, [[128, 128], [128 * 128, 3], [1, 128]])
            )

            # shared_tail: ALL 12 planes' tail rows (496-511) ride ONE early
            # 384KB SWDGE load into a persistent tile (replaces 12
            # 16-partition 32KB loads), and their horizontal adds run in 3
            # big piece-ops during the otherwise-idle DVE startup window —
            # moving ~6.4us of DVE work off the saturated mid-stream and
            # taking group 4 off every plane's drain path.
            if shared_tail:
                xtailt = constp.tile([128, NCH * XTW], F16, name="xtailt")
                atailt = constp.tile([128, NCH * XTW], F16, name="atailt")
                ptailt = constp.tile([128, NCH * XTW], F16, name="ptailt")
                xt_v = xtailt[:].rearrange("p (c f) -> p c f", c=NCH)
                at_v = atailt[:].rearrange("p (c f) -> p c f", c=NCH)
                pt_v = ptailt[:].rearrange("p (c f) -> p c f", c=NCH)

            # Warmup matmul consumes the weight tile right after its DMA.
            wup_ps = psump.tile([128, 512], F32, tag="pg1")
            warm = nc.tensor.matmul(
                wup_ps[0:1, 0 : 3 * 128],
                vt16[0:128, 0:1],
                vt16[:],
                start=True,
                stop=True,
            )
            prev = {"mm": warm, "dve": None, "act": None, "gps": None}

            def chain(inst, which):
                p = prev[which]
                if p is not None:
                    add_dep_helper(inst.ins, p.ins, sync=False, reason=which)
                prev[which] = inst
                return inst

            def chain_dma(inst):
                return chain(inst, "gps")

            # fp32->fp16 conversion happens INSIDE the load DMA (SWDGE casts
            # inline). This removes the DVE cast — the only 2-port-mode DVE
            # op — so GpSimd/SWDGE SBUF-port interference with DVE 2-port
            # modes (HW-measured: a 160ns pad copy stretched to 4.4us while
            # a GpSimd op ran) cannot bite: all remaining DVE ops are
            # 1-port fp16 2x_1P.
            # Steady state (planes 0..NCH-3): whole-plane ops — fewest
            # instructions, lowest DVE busy (DVE is the second-binding
            # resource at ~90% mid-kernel). Tail planes (last two): chunked
            # into row groups {0,1} / {2,3,4} so the post-load latency
            # chain is chunk-granular, and the last plane additionally
            # splits its load, alternates evacs Scalar/DVE and fans its
            # stores across the Sync+GpSimd rings — all of which shortens
            # the pipeline drain after the final load byte.
            for c in range(NCH):
                last = c == NCH - 1
                # Last plane: the 16-row tail group is processed FIRST
                # (its tiny load lands right away), so the post-load
                # critical chain is only chunk {2,3} and the out2 store
                # fires early instead of last.
                if tail3 and last:
                    # Final plane: chunk {0,1,2} then {4} then {3}. Loads
                    # are ordered tail16 -> subtiles 0-2 -> subtile 3, so
                    # the post-last-load critical chain is only group 3's
                    # adds + matmul pair + evac + one 129KB store (~3us)
                    # instead of a 3-group chunk (~6us).
                    chunks = [(0, 3), (4, 1), (3, 1)]
                elif tail_opt and last:
                    chunks = [(4, 1), (0, 2), (2, 2)]
                elif c < whole_planes:
                    # Steady-state planes: whole-plane DVE ops — fewest
                    # instructions, ~0.4us/plane less DVE busy (matters
                    # under chip-wide throttle, where DVE approaches
                    # co-bottleneck with DMA). Tail planes stay chunked
                    # for drain latency.
                    chunks = [(0, 4)] if shared_tail else [(0, 5)]
                elif shared_tail:
                    chunks = [(0, 2), (2, 2)]
                else:
                    chunks = [(0, 2), (2, 3)]
                if shared_tail and c == 0:
                    chain_dma(
                        nc.gpsimd.dma_start(
                            xt_v[0:16, 0:NCH, PAD : PAD + W],
                            _mk_ap(
                                img[:], (H - 16) * W, [[W, 16], [H * W, NCH], [1, W]]
                            ),
                        )
                    )
                if shared_tail and c % 4 == 0:
                    # Tail-piece horizontal adds for planes c..c+3 — FD-bound
                    # on 16 partitions; piece 0 runs in the idle DVE window
                    # before plane 0's main load even lands.
                    cs, ce = c, c + 4
                    chain(
                        nc.vector.tensor_copy(
                            xt_v[0:16, cs:ce, 0:2], xt_v[0:16, cs:ce, 4:2:-1]
                        ),
                        "dve",
                    )
                    chain(
                        nc.vector.tensor_copy(
                            xt_v[0:16, cs:ce, XTW - 2 : XTW],
                            xt_v[0:16, cs:ce, XTW - 4 : XTW - 6 : -1],
                        ),
                        "dve",
                    )
                    chain(
                        nc.vector.tensor_tensor(
                            at_v[0:16, cs:ce, 0 : XTW - 2],
                            xt_v[0:16, cs:ce, 0 : XTW - 2],
                            xt_v[0:16, cs:ce, 2:XTW],
                            mybir.AluOpType.add,
                        ),
                        "dve",
                    )
                    chain(
                        nc.vector.tensor_tensor(
                            pt_v[0:16, cs:ce, 0:W],
                            at_v[0:16, cs:ce, 0:W],
                            xt_v[0:16, cs:ce, 4 : 4 + W],
                            mybir.AluOpType.add,
                        ),
                        "dve",
                    )
                # Casting loads (SWDGE). The group bases (0,124,248,372)
                # are uniform stride-124, so one 3-D AP covers the 4 main
                # subtiles: one DIRECT2D (~0.7us SWDGE desc-gen) per plane
                # instead of four — desc-gen no longer paces the 36us load
                # stream. (A 2-plane-per-DMA variant is impossible: the
                # src AP would need 4 dims — plane stride 512W is not a
                # multiple of the 124W subtile stride — and DMA APs are
                # capped at 3 dims.)
                xh = xhp.tile([128, NSUB * XTW], F16, tag="xh")
                xh3 = xh[:].rearrange("p (a f) -> p a f", a=NSUB)
                if not last:
                    chain_dma(
                        nc.gpsimd.dma_start(
                            xh3[:, 0:4, PAD : PAD + W],
                            _mk_ap(
                                img[:], c * H * W, [[W, 128], [124 * W, 4], [1, W]]
                            ),
                        )
                    )
                    if not shared_tail:
                        chain_dma(
                            nc.gpsimd.dma_start(
                                xh3[0:16, 4, PAD : PAD + W], img[c, H - 16 : H, :]
                            )
                        )
                elif shared_tail:
                    # Last plane: main load split in two so chunk-A compute
                    # overlaps the chunk-B load (tail rows already on chip).
                    chain_dma(
                        nc.gpsimd.dma_start(
                            xh3[:, 0:2, PAD : PAD + W],
                            _mk_ap(
                                img[:], c * H * W, [[W, 128], [124 * W, 2], [1, W]]
                            ),
                        )
                    )
                    chain_dma(
                        nc.gpsimd.dma_start(
                            xh3[:, 2:4, PAD : PAD + W],
                            _mk_ap(
                                img[:],
                                (c * H + 248) * W,
                                [[W, 128], [124 * W, 2], [1, W]],
                            ),
                        )
                    )
                elif tail3:
                    chain_dma(
                        nc.gpsimd.dma_start(
                            xh3[0:16, 4, PAD : PAD + W], img[c, H - 16 : H, :]
                        )
                    )
                    chain_dma(
                        nc.gpsimd.dma_start(
                            xh3[:, 0:3, PAD : PAD + W],
                            _mk_ap(
                                img[:], c * H * W, [[W, 128], [124 * W, 3], [1, W]]
                            ),
                        )
                    )
                    chain_dma(
                        nc.gpsimd.dma_start(
                            xh3[:, 3:4, PAD : PAD + W],
                            _mk_ap(
                                img[:],
                                (c * H + 372) * W,
                                [[W, 128], [124 * W, 1], [1, W]],
                            ),
                        )
                    )
                else:
                    # Last plane: tail rows first, then the main load split
                    # in two, so chunk-A compute overlaps the chunk-B load.
                    chain_dma(
                        nc.gpsimd.dma_start(
                            xh3[0:16, 4, PAD : PAD + W], img[c, H - 16 : H, :]
                        )
                    )
                    chain_dma(
                        nc.gpsimd.dma_start(
                            xh3[:, 0:2, PAD : PAD + W],
                            _mk_ap(
                                img[:], c * H * W, [[W, 128], [124 * W, 2], [1, W]]
                            ),
                        )
                    )
                    chain_dma(
                        nc.gpsimd.dma_start(
                            xh3[:, 2:4, PAD : PAD + W],
                            _mk_ap(
                                img[:],
                                (c * H + 248) * W,
                                [[W, 128], [124 * W, 2], [1, W]],
                            ),
                        )
                    )

                at = xhp.tile([128, NSUB * XTW], F16, tag="a")
                a3 = at[:].rearrange("p (a f) -> p a f", a=NSUB)
                pt = xhp.tile([128, NSUB * XTW], F16, tag="p")
                p3 = pt[:].rearrange("p (a f) -> p a f", a=NSUB)
                ot = outp.tile([128, NSUB * W], F16)
                ot3 = ot[:].rearrange("p (g f) -> p g f", g=NSUB)

                pgs = [
                    psump.tile([128, W], F32, tag="pg1", name="pg1t")
                    for _ in range(5)
                ]
                if shared_tail:
                    # Group 4 (tail rows) first: its inputs were ready at
                    # startup, so its matmul/evac/out2-store never sit on
                    # the drain path.
                    chain(
                        nc.tensor.matmul(
                            pgs[4][0:128, :],
                            vt16r[0:16, 2, 0:128],
                            at_v[0:16, c, 1 : 1 + W],
                            start=True,
                            stop=False,
                        ),
                        "mm",
                    )
                    chain(
                        nc.tensor.matmul(
                            pgs[4][0:128, :],
                            vt16r[0:16, 2, 0:128],
                            pt_v[0:16, c, 0:W],
                            start=False,
                            stop=True,
                        ),
                        "mm",
                    )
                    if merged_tail:
                        evac4_dst = otail[0:128, c * W : (c + 1) * W]
                    else:
                        evac4_dst = ot3[0:128, 4, :]
                    chain(
                        nc.scalar.mul(evac4_dst, pgs[4][0:128, :], INV_AREA),
                        "act",
                    )
                    if not merged_tail:
                        nc.sync.dma_start(out2[c], ot3[0:14, 4, :])
                for g0, ng in chunks:
                    g1 = g0 + ng
                    # Reflect-pad columns on f16 for this chunk's subtiles:
                    # f 0,1 <- f 4,3 ; 514,515 <- 512,511
                    chain(
                        nc.vector.tensor_copy(
                            xh3[:, g0:g1, 0:2], xh3[:, g0:g1, 4:2:-1]
                        ),
                        "dve",
                    )
                    chain(
                        nc.vector.tensor_copy(
                            xh3[:, g0:g1, XTW - 2 : XTW],
                            xh3[:, g0:g1, XTW - 4 : XTW - 6 : -1],
                        ),
                        "dve",
                    )
                    # A[w] = Xp[w] + Xp[w+2]: fp16 2x packed (4B-aligned)
                    chain(
                        nc.vector.tensor_tensor(
                            a3[:, g0:g1, 0 : XTW - 2],
                            xh3[:, g0:g1, 0 : XTW - 2],
                            xh3[:, g0:g1, 2:XTW],
                            mybir.AluOpType.add,
                        ),
                        "dve",
                    )
                    # P[w] = A[w] + Xp[w+4] (aligned fp16 2x). The taps
                    # A[w+1] go straight to the PE as a second accumulating
                    # matmul — the PE has no rhs alignment penalty.
                    chain(
                        nc.vector.tensor_tensor(
                            p3[:, g0:g1, 0:W],
                            a3[:, g0:g1, 0:W],
                            xh3[:, g0:g1, 4 : 4 + W],
                            mybir.AluOpType.add,
                        ),
                        "dve",
                    )
                    # A<<1 matmuls first (their input lands before P), so
                    # the PE burst starts earlier and overlaps P's DVE time.
                    for g in range(g0, g1):
                        vi = _VM_IDX[g]
                        kk = GROUPS[g][1]
                        chain(
                            nc.tensor.matmul(
                                pgs[g][0:128, :],
                                vt16r[0:kk, vi, 0:128],
                                a3[0:kk, g, 1 : 1 + W],
                                start=True,
                                stop=False,
                            ),
                            "mm",
                        )
                    for g in range(g0, g1):
                        vi = _VM_IDX[g]
                        kk = GROUPS[g][1]
                        chain(
                            nc.tensor.matmul(
                                pgs[g][0:128, :],
                                vt16r[0:kk, vi, 0:128],
                                p3[0:kk, g, 0:W],
                                start=False,
                                stop=True,
                            ),
                            "mm",
                        )
                        # Evacuate PSUM -> SBUF f16 with the 1/25 scale.
                        # With merged_tail the g4 evac lands in a persistent
                        # accumulation tile, stored once at kernel end —
                        # drops 11 tiny out2 stores (~0.8us Sync desc-gen
                        # each for 14KB of data).
                        if merged_tail and g == 4:
                            evac_dst = otail[0:128, c * W : (c + 1) * W]
                        else:
                            evac_dst = ot3[0:128, g, :]
                        chain(
                            nc.scalar.mul(evac_dst, pgs[g][0:128, :], INV_AREA),
                            "act",
                        )

                # Stores: loads own the SWDGE ring, stores the Sync HWDGE
                # ring, so the streams overlap. Steady state uses one merged
                # out1 store (fewest DMAs); the last plane goes chunk-
                # granular and fans across Sync + GpSimd (idle by then) so
                # the two desc-gens (~0.8us each) run in parallel.
                if tail3 and last:
                    # Sync FIFO order matches evac completion order:
                    # groups 0-2, then out2 (g4), then the final g3 slice.
                    nc.sync.dma_start(
                        _mk_ap(
                            out1[:], c * 126 * 4 * W, [[4 * W, 126], [1, 3 * W]]
                        ),
                        ot[0:126, 0 : 3 * W],
                    )
                    if not merged_tail:
                        nc.sync.dma_start(out2[c], ot3[0:14, 4, :])
                    nc.sync.dma_start(
                        _mk_ap(
                            out1[:],
                            c * 126 * 4 * W + 3 * W,
                            [[4 * W, 126], [1, W]],
                        ),
                        ot3[0:126, 3, :],
                    )
                else:
                    if tail_opt and last and not merged_tail:
                        # out2 (g4, done first) then chunk A then chunk B.
                        nc.sync.dma_start(out2[c], ot3[0:14, 4, :])
                    nc.sync.dma_start(
                        _mk_ap(
                            out1[:], c * 126 * 4 * W, [[4 * W, 126], [1, 2 * W]]
                        ),
                        ot[0:126, 0 : 2 * W],
                    )
                    nc.sync.dma_start(
                        _mk_ap(
                            out1[:],
                            c * 126 * 4 * W + 2 * W,
                            [[4 * W, 126], [1, 2 * W]],
                        ),
                        ot3[0:126, 2:4, :],
                    )
                    if (
                        not merged_tail
                        and not shared_tail
                        and not (tail_opt and last)
                    ):
                        nc.sync.dma_start(out2[c], ot3[0:14, 4, :])
            if merged_tail:
                nc.sync.dma_start(out2[:], otail[0:14, :])

    if split_waits:
        _split_waits(nc)
    return nc


def _split_waits(nc):
    """Walrus legalization: each 64B ISA instruction has ONE sync-wait slot.

    Tile emits instructions with multiple semaphore waits; split the extras
    into standalone InstEventSemaphore sequencer waits (same engine queue,
    immediately before the instruction) which is semantically identical.
    """
    for fn in nc.m.functions:
        for b in fn.blocks:
            insts = b.instructions
            if not any(
                ins.sync_info and len(ins.sync_info.on_wait) > 1 for ins in insts
            ):
                continue
            new = []
            for ins in insts:
                si = ins.sync_info
                if si and len(si.on_wait) > 1:
                    waits = list(si.on_wait)
                    for w in waits[:-1]:
                        ev = mybir.InstEventSemaphore(
                            name=nc.get_next_instruction_name(),
                            engine=ins.engine,
                            ins=[],
                            outs=[],
                        )
                        ev.sync_info = mybir.SyncInfo(on_wait=[w], on_update=[])
                        new.append(ev)
                    si.on_wait = [waits[-1]]
                new.append(ins)
            b.instructions = new


_NC_CACHE = {}


def _get_module(**kw):
    key = tuple(sorted(kw.items()))
    if key not in _NC_CACHE:
        _NC_CACHE[key] = build_module(**kw)
    return _NC_CACHE[key]


def kernel(image, _trace=False, _trace_kwargs=None, **_variant):
    image = np.asarray(image)
    assert image.shape == (NB, 3, H, W), image.shape
    in_dtype = image.dtype
    image = np.ascontiguousarray(image.astype(np.float32, copy=False))

    nc = _get_module(**_variant)
    in_maps = [
        {
            "image": image[i * NBPC : (i + 1) * NBPC].reshape(NCH, H, W),
            "vmats16": VMATS16,
        }
        for i in range(N_CORES)
    ]
    res = run_bass_kernel_spmd(
        nc,
        in_maps,
        list(range(N_CORES)),
        trace=_trace,
        **(_trace_kwargs or {}),
    )
    # Device layout: out1[c, m, g*W + w] holds output row GROUPS[g].out_base + m
    # (g<4); out2 holds the 14-row tail group.
    dev1 = np.concatenate(
        [
            np.asarray(res.results[i]["out1"]).reshape(NBPC, 3, 126, 4 * W)
            for i in range(N_CORES)
        ],
        axis=0,
    )
    if _variant.get("merged_tail"):
        dev2 = np.concatenate(
            [
                np.asarray(res.results[i]["out2"])
                .reshape(14, NCH, W)
                .transpose(1, 0, 2)
                .reshape(NBPC, 3, 14, W)
                for i in range(N_CORES)
            ],
            axis=0,
        )
    else:
        dev2 = np.concatenate(
            [
                np.asarray(res.results[i]["out2"]).reshape(NBPC, 3, 14, W)
                for i in range(N_CORES)
            ],
            axis=0,
        )
    full = np.empty((NB, 3, H, W), np.float32)
    for g, (_, _, ob, m) in enumerate(GROUPS[:4]):
        full[:, :, ob : ob + m, :] = dev1[:, :, 0:m, g * W : (g + 1) * W].astype(
            np.float32
        )
    ob, m = GROUPS[4][2], GROUPS[4][3]
    full[:, :, ob : ob + m, :] = dev2.astype(np.float32)
    out = full.astype(in_dtype, copy=False)
    if _trace:
        return out, res
    return out

